# revision 36
# baseline (speedup 1.0000x reference)
"""Trainium2 Bass kernel for nn_DFHLoss (DFH loss_fn).

Computes, on 8 NeuronCores (data-parallel over num_train):
  - metric loss: mean over [256, 100000] of softplus pairwise terms
  - 200-step inner codebook SGD on V [64,100] (replicated on every core)
  - quantization loss
Returns (loss_scalar, V_new) matching the jax reference.

Math restructuring (validated vs reference, V rel err ~5e-4 across seeds):
  The SGD step folds to W' = W + a*a3*sign(W) + [At_j @ W + (-a*a2)(W S) + CBW]
  with W = V^T, S = V V^T, At_j = A_j - I small (~1e-3), CBW = a*a1*(b@y)^T.
  The fp32 carry (W + ...) rides the DVE add; every matmul term is a small
  correction, so all matmuls run bf16. The bracketed terms consume state
  (Wb/Sb/Vbs/sgn) derived from W_{t-1} (one-step stale), which removes the
  matmul chain from the serial critical path; staleness shifts V_new by
  ~2e-4 relative, far under tolerance.
  The metric elementwise chain folds to softplus((z2'+1)*z1' + 1) with
  z1' = (0.5u)@U, z2' = (-2y)@Y (both bf16: error averages out over 25.6M
  terms, ~4e-7 on the mean; one-hot z2' is exact). The reference's
  clip(ip,-100,50) only binds on s=1 self-pairs where softplus(M-ip)~1e-22,
  numerically irrelevant at fp32, so it is elided. softplus = Ln(Exp(w)+1)
  on the scalar engine (both funcs share one ACT table) with a fused
  per-partition accumulator on the Ln pass.
"""

import numpy as np
import ml_dtypes

_bf16 = ml_dtypes.bfloat16

N_CORES = 8
NUM_TRAIN = 100000
SHARD = NUM_TRAIN // N_CORES  # 12500
HALF = SHARD // 2  # 6250
B = 256
BIT = 64
N_CLASS = 100
MU, M_MARGIN, ETA, VUL, NTA = 1.0, 1.0, 0.5, 1.0, 1.0

A1 = 2.0 / (BIT * B)
A2 = VUL * 4.0 / (N_CLASS * N_CLASS)
A3 = NTA * 2.0 / (BIT * N_CLASS)

ALPHAS = [0.03, 0.003, 0.0003]
N_ITERS = 200
GROUP = 5  # iterations fused per update (validated ~3-4e-3 vs reference)


def _alpha_idx(t):
    if t >= 179:
        return 2
    if t >= 149:
        return 1
    return 0


CHUNK = 512
N_CHUNKS = (HALF + CHUNK - 1) // CHUNK  # 13 (last chunk 106 cols)

_CACHE = {}


def _build():
    import concourse.bacc as bacc
    import concourse.tile as tile
    import concourse.mybir as mybir

    f32 = mybir.dt.float32
    bf16 = mybir.dt.bfloat16
    AF = mybir.ActivationFunctionType
    OP = mybir.AluOpType

    # Force a single ACT function table: the default greedy table chooser
    # ping-pongs between 'exp_and_others' and 'natural_log' (one ~1.3us
    # ACT_TABLE_LOAD per metric tile). Emptying every table except the one
    # containing ALL funcs we use (exp, ln, sign, copy, square, identity)
    # keeps positions (= act_func_set_id) valid while making the chooser
    # always pick it.
    from concourse.hw_specs import get_activation_tables as _orig_gat

    def _one_table(arch):
        tabs = _orig_gat(arch)
        return {
            name: (funcs if name == "natural_log_exp_and_others" else frozenset())
            for name, funcs in tabs.items()
        }

    bacc.get_activation_tables = _one_table

    nc = bacc.Bacc(None, target_bir_lowering=False)

    # inputs (identical on every core except Ush/Ysh)
    d_u05T = nc.declare_dram_parameter("u05T", [128, B], bf16, isOutput=False)
    d_uT = nc.declare_dram_parameter("uT", [BIT, B], f32, isOutput=False)
    d_yT = nc.declare_dram_parameter("yT", [N_CLASS, B], f32, isOutput=False)
    d_yTn2 = nc.declare_dram_parameter("yTn2", [N_CLASS, B], bf16, isOutput=False)
    d_y0 = nc.declare_dram_parameter("y0", [128, N_CLASS], f32, isOutput=False)
    d_y1 = nc.declare_dram_parameter("y1", [128, N_CLASS], f32, isOutput=False)
    d_W0 = nc.declare_dram_parameter("W0", [N_CLASS, BIT], f32, isOutput=False)
    d_W0b = nc.declare_dram_parameter("W0b", [N_CLASS, BIT], bf16, isOutput=False)
    d_SbI0 = nc.declare_dram_parameter("SbI0", [128, BIT], bf16, isOutput=False)
    d_V0bs = nc.declare_dram_parameter("V0bs", [BIT, N_CLASS], bf16, isOutput=False)
    d_G0b = nc.declare_dram_parameter("G0b", [N_CLASS, BIT], bf16, isOutput=False)
    d_Ap = [
        nc.declare_dram_parameter(f"Ap{v}", [N_CLASS, N_CLASS], bf16, isOutput=False)
        for v in range(5)
    ]
    d_I64 = nc.declare_dram_parameter("I64", [BIT, BIT], f32, isOutput=False)
    d_I100b = nc.declare_dram_parameter(
        "I100b", [N_CLASS, N_CLASS], bf16, isOutput=False
    )
    d_I100f = nc.declare_dram_parameter(
        "I100f", [N_CLASS, N_CLASS], f32, isOutput=False
    )
    d_ones = nc.declare_dram_parameter("ones", [128, 1], f32, isOutput=False)
    d_Ush = nc.declare_dram_parameter("Ush", [128, HALF], bf16, isOutput=False)
    d_Ysh = nc.declare_dram_parameter("Ysh", [N_CLASS, SHARD], bf16, isOutput=False)

    # outputs
    d_Vout = nc.declare_dram_parameter("V_new", [BIT, N_CLASS], f32, isOutput=True)
    d_part = nc.declare_dram_parameter("partials", [1, 2], f32, isOutput=True)

    prefetched_u = {}
    prefetched_y = {}
    with tile.TileContext(nc) as tc:
        with (
            tc.tile_pool(name="consts", bufs=1) as cpool,
            tc.tile_pool(name="wstate", bufs=3) as vpool,
            tc.tile_pool(name="bstate", bufs=4) as bpool,
            tc.tile_pool(name="vtmp", bufs=2) as tpool,
            tc.tile_pool(name="uchunk", bufs=4) as upool,
            tc.tile_pool(name="ychunk", bufs=4) as ypool,
            tc.tile_pool(name="mscratch", bufs=3) as mpool,
            tc.tile_pool(name="acc", bufs=1) as apool,
            tc.tile_pool(name="ps_z1", bufs=2, space="PSUM") as ps_z1,
            tc.tile_pool(name="ps_z2", bufs=2, space="PSUM") as ps_z2,
            tc.tile_pool(name="ps_w", bufs=2, space="PSUM") as ps_w,
            tc.tile_pool(name="ps_s", bufs=1, space="PSUM") as ps_s,
            tc.tile_pool(name="ps_v", bufs=1, space="PSUM") as ps_v,
        ):
            # ---- prefetch the first metric chunks (sync queues) while the
            # constants load on the gpsimd queues: kills the startup bubble
            for c0 in (0, 1):
                ut = upool.tile([128, CHUNK], bf16, tag="Ut")
                cs = c0 * CHUNK
                for q in range(4):
                    p0, p1 = 32 * q, 32 * (q + 1)
                    nc.sync.dma_start(
                        ut[p0:p1, :], d_Ush[p0:p1, cs : cs + CHUNK]
                    )
                prefetched_u[c0] = ut
                for h0 in (0, 1):
                    yt = ypool.tile([N_CLASS, CHUNK], bf16, tag="Yt")
                    off = h0 * HALF + c0 * CHUNK
                    for q in range(4):
                        p0, p1 = 25 * q, 25 * (q + 1)
                        nc.sync.dma_start(
                            yt[p0:p1, :], d_Ysh[p0:p1, off : off + CHUNK]
                        )
                    prefetched_y[(c0, h0)] = yt

            # ---- load constants ----
            def cload(dram, shape, tag, dt=f32):
                t = cpool.tile(shape, dt, tag=tag)
                nc.gpsimd.dma_start(t[:], dram[:])
                return t

            # order matters: the gpsimd issue thread serializes these, so
            # prelude-critical tensors go first, loop/finalize tensors last
            yT = cload(d_yT, [N_CLASS, B], "yT")
            uT = cload(d_uT, [BIT, B], "uT")
            W0 = cload(d_W0, [N_CLASS, BIT], "W0")
            I64 = cload(d_I64, [BIT, BIT], "I64")
            y0 = cload(d_y0, [128, N_CLASS], "y0")
            y1 = cload(d_y1, [128, N_CLASS], "y1")
            u05T = cload(d_u05T, [128, B], "u05T", bf16)
            yTn2 = cload(d_yTn2, [N_CLASS, B], "yTn2", bf16)
            W0b = cload(d_W0b, [N_CLASS, BIT], "W0b", bf16)
            G0b = cload(d_G0b, [N_CLASS, BIT], "G0b", bf16)
            Apb = [cload(d_Ap[v], [N_CLASS, N_CLASS], f"Ap{v}", bf16) for v in range(5)]
            I100b = cload(d_I100b, [N_CLASS, N_CLASS], "I100b", bf16)
            I100f = cload(d_I100f, [N_CLASS, N_CLASS], "I100f")
            ones = cload(d_ones, [128, 1], "ones")
            # ping-pong merged operands: SbI = [Sb ; I64], VbsCB = [Vbs ; CBVb]
            SbI = []
            VbsCB = []
            for p in range(2):
                t = cpool.tile([128, BIT], bf16, tag=f"SbI{p}")
                nc.gpsimd.dma_start(t[:], d_SbI0[:])
                SbI.append(t)
                t2 = cpool.tile([128, N_CLASS], bf16, tag=f"VbsCB{p}")
                nc.gpsimd.dma_start(t2[0:BIT, :], d_V0bs[:])
                VbsCB.append(t2)
         # ---- prelude: b = sign(C @ yT + uT), B = b@y, CBV (bf16) ----
            sgW0 = cpool.tile([N_CLASS, BIT], f32, tag="sgW0")
            nc.scalar.activation(sgW0[:], W0[:], AF.Sign)
            b_ps = ps_z1.tile([BIT, B], f32, tag="z1")
            nc.tensor.matmul(b_ps[:], sgW0[:], yT[:], start=True, stop=True)
            badd = cpool.tile([BIT, B], f32, tag="badd")
            nc.vector.tensor_tensor(badd[:], b_ps[:], uT[:], OP.add)
            b_sb = cpool.tile([BIT, B], f32, tag="b_sb")
            nc.scalar.activation(b_sb[:], badd[:], AF.Sign)

            # quant: sum((b - uT)^2)
            qd = cpool.tile([BIT, B], f32, tag="qd")
            nc.vector.tensor_tensor(qd[:], b_sb[:], uT[:], OP.subtract)
            qsq = cpool.tile([BIT, B], f32, tag="qsq")
            qcol = cpool.tile([BIT, 1], f32, tag="qcol")
            nc.scalar.activation(qsq[:], qd[:], AF.Square, accum_out=qcol[:])

            # B = b @ y via transposed halves; CBVb_j = a_j*a1*B in bf16
            bT_ps = ps_z2.tile([128, BIT], f32, tag="z2")
            nc.tensor.transpose(bT_ps[:], b_sb[:, 0:128], I64[:])
            bT0 = cpool.tile([128, BIT], f32, tag="bT0")
            nc.scalar.copy(bT0[:], bT_ps[:])
            bT_ps2 = ps_z2.tile([128, BIT], f32, tag="z2")
            nc.tensor.transpose(bT_ps2[:], b_sb[:, 128:256], I64[:])
            bT1 = cpool.tile([128, BIT], f32, tag="bT1")
            nc.scalar.copy(bT1[:], bT_ps2[:])

            B_ps = ps_z1.tile([BIT, N_CLASS], f32, tag="z1")
            nc.tensor.matmul(B_ps[:], bT0[:], y0[:], start=True, stop=False)
            nc.tensor.matmul(B_ps[:], bT1[:], y1[:], start=False, stop=True)
            # CB half of the merged operand: s_al*a1*B - s_al*a3*ones in
            # CBVb-form; the K=128 merged matmul (VbsCB^T @ SbI) then yields
            # cubic + CB^T, carrying both the b@y term and the sign-trick
            # offset (sign(W) = 2*(W>0) - 1) for the whole pair.
            B_sb = cpool.tile([BIT, N_CLASS], f32, tag="B_sb")
            nc.scalar.copy(B_sb[:], B_ps[:])

            def build_cb(tile_, s_al):
                nc.scalar.activation(
                    tile_[BIT:128, :], B_sb[:], AF.Copy,
                    bias=-s_al * A3, scale=s_al * A1,
                )

            # ---- metric accumulator ----
            NT = 2 * 2 * N_CHUNKS  # 52 softplus tiles
            acc = apool.tile([128, NT], f32, tag="acc")

            metric_units = []
            for c in range(N_CHUNKS):
                cw = min(CHUNK, HALF - c * CHUNK)
                for h in range(2):
                    for m in range(2):
                        metric_units.append((c, h, m, cw))
            mu_state = {}
            stageA_done = [0]
            stageB_done = [0]
            acc_idx = [0]

            def emit_A(i):
                c, h, m, cw = metric_units[i]
                if m == 0 and h == 0:
                    if c in prefetched_u:
                        mu_state[("U", c)] = prefetched_u.pop(c)
                    else:
                        ut = upool.tile([128, CHUNK], bf16, tag="Ut")
                        cs = c * CHUNK
                        nc.sync.dma_start(
                            ut[0:64, 0:cw], d_Ush[0:64, cs : cs + cw]
                        )
                        nc.sync.dma_start(
                            ut[64:128, 0:cw], d_Ush[64:128, cs : cs + cw]
                        )
                        mu_state[("U", c)] = ut
                if m == 0:
                    if (c, h) in prefetched_y:
                        mu_state[("Y", c, h)] = prefetched_y.pop((c, h))
                    else:
                        yt = ypool.tile([N_CLASS, CHUNK], bf16, tag="Yt")
                        off = h * HALF + c * CHUNK
                        nc.sync.dma_start(
                            yt[0:50, 0:cw], d_Ysh[0:50, off : off + cw]
                        )
                        nc.sync.dma_start(
                            yt[50:100, 0:cw], d_Ysh[50:100, off : off + cw]
                        )
                        mu_state[("Y", c, h)] = yt
                ut = mu_state[("U", c)]
                yt = mu_state[("Y", c, h)]
                z1 = ps_z1.tile([128, CHUNK], f32, tag="z1")
                nc.tensor.matmul(
                    z1[:, 0:cw],
                    u05T[h * BIT : (h + 1) * BIT, m * 128 : (m + 1) * 128],
                    ut[h * BIT : (h + 1) * BIT, 0:cw],
                    start=True,
                    stop=True,
                )
                z2 = ps_z2.tile([128, CHUNK], f32, tag="z2")
                nc.tensor.matmul(
                    z2[:, 0:cw],
                    yTn2[:, m * 128 : (m + 1) * 128],
                    yt[:, 0:cw],
                    start=True,
                    stop=True,
                )
                c1 = mpool.tile([128, CHUNK], f32, tag="c1")
                if i % 3 == 2:
                    nc.vector.tensor_copy(c1[:, 0:cw], z1[:, 0:cw])
                else:
                    nc.scalar.copy(c1[:, 0:cw], z1[:, 0:cw])
                mu_state[("AB", i)] = (z1, z2, c1, cw)

            BATCH = 6
            wbig = {"t": None, "n": 0}

            def flush_wbig():
                # one wide Exp+Ln amortizes the ~352-cycle ACT pipeline fill
                n = wbig["n"]
                if n == 0:
                    return
                wt = wbig["t"]
                ew = mpool.tile([128, BATCH * CHUNK], f32, tag="ew")
                nc.scalar.activation(
                    ew[:, 0 : n * CHUNK], wt[:, 0 : n * CHUNK], AF.Exp,
                    bias=M_MARGIN,
                )
                sp = mpool.tile([128, BATCH * CHUNK], f32, tag="sp")
                i2 = acc_idx[0]
                nc.scalar.activation(
                    sp[:, 0 : n * CHUNK],
                    ew[:, 0 : n * CHUNK],
                    AF.Ln,
                    bias=1.0,
                    accum_out=acc[:, i2 : i2 + 1],
                )
                acc_idx[0] += 1
                wbig["t"] = None
                wbig["n"] = 0

            def emit_B(i):
                z1, z2, c1, cw = mu_state.pop(("AB", i))
                if cw == CHUNK:
                    if wbig["t"] is None:
                        wbig["t"] = mpool.tile([128, BATCH * CHUNK], f32, name="wb", tag="wb")
                    wt = wbig["t"]
                    o = wbig["n"] * CHUNK
                    nc.vector.scalar_tensor_tensor(
                        wt[:, o : o + cw], z2[:, 0:cw], 1.0, c1[:, 0:cw],
                        OP.add, OP.mult,
                    )
                    wbig["n"] += 1
                    if wbig["n"] == BATCH:
                        flush_wbig()
                    return
                flush_wbig()
                w = mpool.tile([128, CHUNK], f32, tag="w")
                nc.vector.scalar_tensor_tensor(
                    w[:, 0:cw], z2[:, 0:cw], 1.0, c1[:, 0:cw], OP.add, OP.mult
                )
                ew = mpool.tile([128, CHUNK], f32, tag="ew")
                nc.scalar.activation(ew[:, 0:cw], w[:, 0:cw], AF.Exp, bias=M_MARGIN)
                sp = mpool.tile([128, CHUNK], f32, tag="sp")
                i2 = acc_idx[0]
                nc.scalar.activation(
                    sp[:, 0:cw],
                    ew[:, 0:cw],
                    AF.Ln,
                    bias=1.0,
                    accum_out=acc[:, i2 : i2 + 1],
                )
                acc_idx[0] += 1

            NU = len(metric_units)

            def emit_metric(frac):
                # keep stage A one unit ahead of stage B
                wantA = min(NU, int(frac * NU) + 1)
                while stageA_done[0] < wantA:
                    emit_A(stageA_done[0])
                    stageA_done[0] += 1
                wantB = min(NU, stageA_done[0] - 1, int(frac * NU))
                while stageB_done[0] < wantB:
                    emit_B(stageB_done[0])
                    stageB_done[0] += 1

            def finish_metric():
                while stageA_done[0] < NU:
                    emit_A(stageA_done[0])
                    stageA_done[0] += 1
                while stageB_done[0] < NU:
                    emit_B(stageB_done[0])
                    stageB_done[0] += 1
                flush_wbig()

            # ---- V loop: fused iteration groups ----
            # Group k applies iters [G*k, G*k+G) in one update using state two
            # groups back: W' = W + [Ap_k @ Wb + VbsCB^T @ SbI + I @ g], where
            # the group's summed alphas are baked into the operands.
            NPAIR = N_ITERS // GROUP
            pj = [
                tuple(_alpha_idx(GROUP * k + i) for i in range(GROUP))
                for k in range(NPAIR)
            ]
            variants = []
            var_of = []
            for k in range(NPAIR):
                if pj[k] not in variants:
                    variants.append(pj[k])
                var_of.append(variants.index(pj[k]))
            assert len(variants) <= 5

            def s_alpha(k):
                return sum(ALPHAS[j] for j in pj[k])

            # prelude: CB halves for the initial variant
            build_cb(VbsCB[0], s_alpha(0))
            build_cb(VbsCB[1], s_alpha(1))
            cb_cur = [var_of[0], var_of[1]]

            W = W0
            states = {0: (W0b, G0b), 1: (W0b, G0b)}
            for k in range(NPAIR):
                s_al = s_alpha(k)
                par = k % 2
                Wb_s, g_s = states.pop(k)

                Wp = ps_w.tile([N_CLASS, BIT], f32, tag="Wp")
                nc.tensor.matmul(
                    Wp[:], Apb[var_of[k]][:], Wb_s[:], start=True, stop=False
                )
                nc.tensor.matmul(
                    Wp[:], VbsCB[par][:], SbI[par][:], start=False, stop=False
                )
                # sign term: g_s holds {0, 2*s_a3*a3-ish} pre-scaled; I-inject
                nc.tensor.matmul(
                    Wp[:], I100b[:], g_s[:], start=False, stop=True
                )

                Wn = vpool.tile([N_CLASS, BIT], f32, tag="W")
                nc.vector.scalar_tensor_tensor(
                    Wn[:], Wp[:], 1.0, W[:], OP.bypass, OP.add
                )
                W = Wn

                # refresh state + merged operands for pair k+2 (same parity)
                if k + 2 < NPAIR:
                    Wb2 = bpool.tile([N_CLASS, BIT], bf16, tag="Wb")
                    nc.gpsimd.tensor_copy(Wb2[:], Wn[:])
                    g2 = bpool.tile([N_CLASS, BIT], bf16, tag="g")
                    nc.vector.tensor_scalar(
                        g2[:], Wn[:], 0.0, 2.0 * s_alpha(k + 2) * A3,
                        OP.is_gt, OP.mult,
                    )
                    S_ps = ps_s.tile([BIT, BIT], f32, tag="S")
                    nc.tensor.matmul(S_ps[:], Wb2[:], Wb2[:], start=True, stop=True)
                    nc.vector.tensor_copy(SbI[par][0:BIT, :], S_ps[:])
                    Vt_ps = ps_v.tile([BIT, N_CLASS], bf16, tag="Vt")
                    nc.tensor.transpose(Vt_ps[:], Wb2[:], I100b[:])
                    nc.vector.tensor_scalar(
                        VbsCB[par][0:BIT, :], Vt_ps[:],
                        -s_alpha(k + 2) * A2, None, OP.mult,
                    )
                    if var_of[k + 2] != cb_cur[par]:
                        build_cb(VbsCB[par], s_alpha(k + 2))
                        cb_cur[par] = var_of[k + 2]
                    states[k + 2] = (Wb2, g2)

                # spread metric work across the loop, slightly front-loaded
                # so the last Exp/Ln batches overlap the final V groups
                emit_metric(min(1.0, 1.12 * (k + 1) / NPAIR))

            finish_metric()

            # ---- finalize: V_new = W^T (fp32 transpose) ----
            Vf_ps = ps_v.tile([BIT, N_CLASS], f32, tag="Vt")
            nc.tensor.transpose(Vf_ps[:], W[:], I100f[:])
            Vf = tpool.tile([BIT, N_CLASS], f32, tag="Vf")
            nc.scalar.copy(Vf[:], Vf_ps[:])
            nc.sync.dma_start(d_Vout[:], Vf[:])

            rsum = apool.tile([128, 1], f32, tag="rsum")
            nc.vector.tensor_reduce(rsum[:], acc[:], mybir.AxisListType.X, OP.add)
            part = apool.tile([1, 2], f32, tag="part")
            msum_ps = ps_s.tile([1, 1], f32, tag="S")
            nc.tensor.matmul(msum_ps[:], rsum[:], ones[:], start=True, stop=True)
            nc.scalar.copy(part[:, 0:1], msum_ps[:])
            qsum_ps = ps_s.tile([1, 1], f32, tag="S")
            nc.tensor.matmul(
                qsum_ps[:], qcol[:], ones[0:BIT, :], start=True, stop=True
            )
            nc.scalar.copy(part[:, 1:2], qsum_ps[:])
            nc.sync.dma_start(d_part[:], part[:])

    nc.compile()
    return nc


def _prep_inputs(u, y, ind, U, Y, V, TK):
    u = np.ascontiguousarray(u, dtype=np.float32)
    y = np.ascontiguousarray(y, dtype=np.float32)
    U = np.array(U, dtype=np.float32, copy=True)
    Y = np.array(Y, dtype=np.float32, copy=True)
    V = np.ascontiguousarray(V, dtype=np.float32)
    TK = np.ascontiguousarray(TK, dtype=np.float32)
    ind = np.asarray(ind)

    # scatter the batch into the train buffers (reference's .at[:, ind].set)
    U[:, ind] = u.T
    Y[:, ind] = y.T

    counts = y.sum(axis=0).astype(np.float32)
    eye = np.eye(N_CLASS, dtype=np.float32)

    def atil(j):
        al = ALPHAS[j]
        return -al * A3 * eye - al * A1 * np.diag(counts) + al * A2 * TK

    ngroup = N_ITERS // GROUP
    pj = [
        tuple(_alpha_idx(GROUP * k + i) for i in range(GROUP)) for k in range(ngroup)
    ]
    variants = []
    for k in range(ngroup):
        if pj[k] not in variants:
            variants.append(pj[k])
    Ap = [
        sum(atil(j) for j in js).astype(_bf16) for js in variants
    ]
    while len(Ap) < 5:
        Ap.append(np.zeros((N_CLASS, N_CLASS), dtype=_bf16))

    W0 = np.ascontiguousarray(V.T)
    W0b = W0.astype(_bf16)
    W0bf = W0b.astype(np.float32)
    S0b = (W0bf.T @ W0bf).astype(_bf16)
    SbI0 = np.vstack([S0b, np.eye(BIT, dtype=np.float32).astype(_bf16)])
    s_al0 = sum(ALPHAS[j] for j in pj[0])
    V0bs = (-s_al0 * A2 * W0bf.T).astype(_bf16)
    G0b = ((W0 > 0) * np.float32(2.0 * s_al0 * A3)).astype(_bf16)

    base = {
        "u05T": np.ascontiguousarray(
            np.vstack([0.5 * u.T, 0.5 * u.T]).astype(_bf16)
        ),  # duplicated so the h=1 matmul's lhsT shares rhs's base partition
        "uT": np.ascontiguousarray(u.T),
        "yT": np.ascontiguousarray(y.T),
        "yTn2": np.ascontiguousarray((-2.0 * y.T).astype(_bf16)),
        "y0": np.ascontiguousarray(y[0:128, :]),
        "y1": np.ascontiguousarray(y[128:256, :]),
        "W0": W0,
        "W0b": W0b,
        "SbI0": np.ascontiguousarray(SbI0),
        "V0bs": np.ascontiguousarray(V0bs),
        "G0b": G0b,
        "Ap0": Ap[0],
        "Ap1": Ap[1],
        "Ap2": Ap[2],
        "Ap3": Ap[3],
        "Ap4": Ap[4],
        "I64": np.eye(BIT, dtype=np.float32),
        "I100b": eye.astype(_bf16),
        "I100f": eye,
        "ones": np.ones((128, 1), dtype=np.float32),
    }

    in_maps = []
    for k in range(N_CORES):
        sl = slice(k * SHARD, (k + 1) * SHARD)
        Usl = U[:, sl]  # [64, 12500]
        Ush = Usl.reshape(BIT, 2, HALF).transpose(1, 0, 2).reshape(128, HALF)
        m = dict(base)
        m["Ush"] = np.ascontiguousarray(Ush.astype(_bf16))
        m["Ysh"] = np.ascontiguousarray(Y[:, sl].astype(_bf16))
        in_maps.append(m)
    return in_maps


def kernel(u, y, ind, U, Y, V, TK):
    from concourse.bass_utils import run_bass_kernel_spmd

    if "nc" not in _CACHE:
        _CACHE["nc"] = _build()
    nc = _CACHE["nc"]

    in_maps = _prep_inputs(u, y, ind, U, Y, V, TK)
    res = run_bass_kernel_spmd(nc, in_maps, core_ids=list(range(N_CORES)))

    metric_sum = sum(float(res.results[k]["partials"][0, 0]) for k in range(N_CORES))
    quant_sum = float(res.results[0]["partials"][0, 1])
    loss = metric_sum / (B * NUM_TRAIN) + ETA * quant_sum / (BIT * B)
    V_new = np.asarray(res.results[0]["V_new"], dtype=np.float32)
    return np.float32(loss), V_new


# revision 37
# speedup vs baseline: 1.0078x; 1.0078x over previous
"""Trainium2 Bass kernel for nn_DFHLoss (DFH loss_fn).

Computes, on 8 NeuronCores (data-parallel over num_train):
  - metric loss: mean over [256, 100000] of softplus pairwise terms
  - 200-step inner codebook SGD on V [64,100] (replicated on every core)
  - quantization loss
Returns (loss_scalar, V_new) matching the jax reference.

Math restructuring (validated vs reference, V rel err ~5e-4 across seeds):
  The SGD step folds to W' = W + a*a3*sign(W) + [At_j @ W + (-a*a2)(W S) + CBW]
  with W = V^T, S = V V^T, At_j = A_j - I small (~1e-3), CBW = a*a1*(b@y)^T.
  The fp32 carry (W + ...) rides the DVE add; every matmul term is a small
  correction, so all matmuls run bf16. The bracketed terms consume state
  (Wb/Sb/Vbs/sgn) derived from W_{t-1} (one-step stale), which removes the
  matmul chain from the serial critical path; staleness shifts V_new by
  ~2e-4 relative, far under tolerance.
  The metric elementwise chain folds to softplus((z2'+1)*z1' + 1) with
  z1' = (0.5u)@U, z2' = (-2y)@Y (both bf16: error averages out over 25.6M
  terms, ~4e-7 on the mean; one-hot z2' is exact). The reference's
  clip(ip,-100,50) only binds on s=1 self-pairs where softplus(M-ip)~1e-22,
  numerically irrelevant at fp32, so it is elided. softplus = Ln(Exp(w)+1)
  on the scalar engine (both funcs share one ACT table) with a fused
  per-partition accumulator on the Ln pass.
"""

import numpy as np
import ml_dtypes

_bf16 = ml_dtypes.bfloat16

N_CORES = 8
NUM_TRAIN = 100000
SHARD = NUM_TRAIN // N_CORES  # 12500
HALF = SHARD // 2  # 6250
B = 256
BIT = 64
N_CLASS = 100
MU, M_MARGIN, ETA, VUL, NTA = 1.0, 1.0, 0.5, 1.0, 1.0

A1 = 2.0 / (BIT * B)
A2 = VUL * 4.0 / (N_CLASS * N_CLASS)
A3 = NTA * 2.0 / (BIT * N_CLASS)

ALPHAS = [0.03, 0.003, 0.0003]
N_ITERS = 200
GROUP = 5  # iterations fused per update (validated ~3-4e-3 vs reference)


def _alpha_idx(t):
    if t >= 179:
        return 2
    if t >= 149:
        return 1
    return 0


CHUNK = 512
N_CHUNKS = (HALF + CHUNK - 1) // CHUNK  # 13 (last chunk 106 cols)

_CACHE = {}


def _build():
    import concourse.bacc as bacc
    import concourse.tile as tile
    import concourse.mybir as mybir

    f32 = mybir.dt.float32
    bf16 = mybir.dt.bfloat16
    AF = mybir.ActivationFunctionType
    OP = mybir.AluOpType

    # Force a single ACT function table: the default greedy table chooser
    # ping-pongs between 'exp_and_others' and 'natural_log' (one ~1.3us
    # ACT_TABLE_LOAD per metric tile). Emptying every table except the one
    # containing ALL funcs we use (exp, ln, sign, copy, square, identity)
    # keeps positions (= act_func_set_id) valid while making the chooser
    # always pick it.
    from concourse.hw_specs import get_activation_tables as _orig_gat

    def _one_table(arch):
        tabs = _orig_gat(arch)
        return {
            name: (funcs if name == "natural_log_exp_and_others" else frozenset())
            for name, funcs in tabs.items()
        }

    bacc.get_activation_tables = _one_table

    nc = bacc.Bacc(None, target_bir_lowering=False)

    # inputs (identical on every core except Ush/Ysh)
    d_u05T = nc.declare_dram_parameter("u05T", [128, B], bf16, isOutput=False)
    d_uT = nc.declare_dram_parameter("uT", [BIT, B], f32, isOutput=False)
    d_yT = nc.declare_dram_parameter("yT", [N_CLASS, B], f32, isOutput=False)
    d_yTn2 = nc.declare_dram_parameter("yTn2", [N_CLASS, B], bf16, isOutput=False)
    d_y0 = nc.declare_dram_parameter("y0", [128, N_CLASS], f32, isOutput=False)
    d_y1 = nc.declare_dram_parameter("y1", [128, N_CLASS], f32, isOutput=False)
    d_W0 = nc.declare_dram_parameter("W0", [N_CLASS, BIT], f32, isOutput=False)
    d_W0b = nc.declare_dram_parameter("W0b", [N_CLASS, BIT], bf16, isOutput=False)
    d_SbI0 = nc.declare_dram_parameter("SbI0", [128, BIT], bf16, isOutput=False)
    d_V0bs = nc.declare_dram_parameter("V0bs", [BIT, N_CLASS], bf16, isOutput=False)
    d_G0b = nc.declare_dram_parameter("G0b", [N_CLASS, BIT], bf16, isOutput=False)
    d_Ap = [
        nc.declare_dram_parameter(f"Ap{v}", [N_CLASS, N_CLASS], bf16, isOutput=False)
        for v in range(5)
    ]
    d_I64 = nc.declare_dram_parameter("I64", [BIT, BIT], f32, isOutput=False)
    d_I100b = nc.declare_dram_parameter(
        "I100b", [N_CLASS, N_CLASS], bf16, isOutput=False
    )
    d_I100f = nc.declare_dram_parameter(
        "I100f", [N_CLASS, N_CLASS], f32, isOutput=False
    )
    d_ones = nc.declare_dram_parameter("ones", [128, 1], f32, isOutput=False)
    d_Ush = nc.declare_dram_parameter("Ush", [128, HALF], bf16, isOutput=False)
    d_Ysh = nc.declare_dram_parameter("Ysh", [N_CLASS, SHARD], bf16, isOutput=False)

    # outputs
    d_Vout = nc.declare_dram_parameter("V_new", [BIT, N_CLASS], f32, isOutput=True)
    d_part = nc.declare_dram_parameter("partials", [1, 2], f32, isOutput=True)

    prefetched_u = {}
    prefetched_y = {}
    with tile.TileContext(nc) as tc:
        with (
            tc.tile_pool(name="consts", bufs=1) as cpool,
            tc.tile_pool(name="wstate", bufs=3) as vpool,
            tc.tile_pool(name="bstate", bufs=4) as bpool,
            tc.tile_pool(name="vtmp", bufs=2) as tpool,
            tc.tile_pool(name="uchunk", bufs=4) as upool,
            tc.tile_pool(name="ychunk", bufs=4) as ypool,
            tc.tile_pool(name="mscratch", bufs=3) as mpool,
            tc.tile_pool(name="acc", bufs=1) as apool,
            tc.tile_pool(name="ps_z1", bufs=2, space="PSUM") as ps_z1,
            tc.tile_pool(name="ps_z2", bufs=2, space="PSUM") as ps_z2,
            tc.tile_pool(name="ps_w", bufs=2, space="PSUM") as ps_w,
            tc.tile_pool(name="ps_s", bufs=1, space="PSUM") as ps_s,
            tc.tile_pool(name="ps_v", bufs=1, space="PSUM") as ps_v,
        ):
            # ---- prefetch the first metric chunks (sync queues) while the
            # constants load on the gpsimd queues: kills the startup bubble
            for c0 in (0, 1):
                ut = upool.tile([128, CHUNK], bf16, tag="Ut")
                cs = c0 * CHUNK
                for q in range(4):
                    p0, p1 = 32 * q, 32 * (q + 1)
                    nc.sync.dma_start(
                        ut[p0:p1, :], d_Ush[p0:p1, cs : cs + CHUNK]
                    )
                prefetched_u[c0] = ut
                for h0 in (0, 1):
                    yt = ypool.tile([N_CLASS, CHUNK], bf16, tag="Yt")
                    off = h0 * HALF + c0 * CHUNK
                    for q in range(4):
                        p0, p1 = 25 * q, 25 * (q + 1)
                        nc.sync.dma_start(
                            yt[p0:p1, :], d_Ysh[p0:p1, off : off + CHUNK]
                        )
                    prefetched_y[(c0, h0)] = yt

            # ---- load constants ----
            def cload(dram, shape, tag, dt=f32):
                t = cpool.tile(shape, dt, tag=tag)
                nc.gpsimd.dma_start(t[:], dram[:])
                return t

            # order matters: the gpsimd issue thread serializes these, so
            # prelude-critical tensors go first, loop/finalize tensors last
            yT = cload(d_yT, [N_CLASS, B], "yT")
            uT = cload(d_uT, [BIT, B], "uT")
            W0 = cload(d_W0, [N_CLASS, BIT], "W0")
            I64 = cload(d_I64, [BIT, BIT], "I64")
            y0 = cload(d_y0, [128, N_CLASS], "y0")
            y1 = cload(d_y1, [128, N_CLASS], "y1")
            u05T = cload(d_u05T, [128, B], "u05T", bf16)
            yTn2 = cload(d_yTn2, [N_CLASS, B], "yTn2", bf16)
            W0b = cload(d_W0b, [N_CLASS, BIT], "W0b", bf16)
            G0b = cload(d_G0b, [N_CLASS, BIT], "G0b", bf16)
            Apb = [cload(d_Ap[v], [N_CLASS, N_CLASS], f"Ap{v}", bf16) for v in range(5)]
            I100b = cload(d_I100b, [N_CLASS, N_CLASS], "I100b", bf16)
            I100f = cload(d_I100f, [N_CLASS, N_CLASS], "I100f")
            ones = cload(d_ones, [128, 1], "ones")
            # ping-pong merged operands: SbI = [Sb ; I64], VbsCB = [Vbs ; CBVb]
            SbI = []
            VbsCB = []
            for p in range(2):
                t = cpool.tile([128, BIT], bf16, tag=f"SbI{p}")
                nc.gpsimd.dma_start(t[:], d_SbI0[:])
                SbI.append(t)
                t2 = cpool.tile([128, N_CLASS], bf16, tag=f"VbsCB{p}")
                nc.gpsimd.dma_start(t2[0:BIT, :], d_V0bs[:])
                VbsCB.append(t2)
         # ---- prelude: b = sign(C @ yT + uT), B = b@y, CBV (bf16) ----
            sgW0 = cpool.tile([N_CLASS, BIT], f32, tag="sgW0")
            nc.scalar.activation(sgW0[:], W0[:], AF.Sign)
            b_ps = ps_z1.tile([BIT, B], f32, tag="z1")
            nc.tensor.matmul(b_ps[:], sgW0[:], yT[:], start=True, stop=True)
            badd = cpool.tile([BIT, B], f32, tag="badd")
            nc.vector.tensor_tensor(badd[:], b_ps[:], uT[:], OP.add)
            b_sb = cpool.tile([BIT, B], f32, tag="b_sb")
            nc.scalar.activation(b_sb[:], badd[:], AF.Sign)

            # quant: sum((b - uT)^2)
            qd = cpool.tile([BIT, B], f32, tag="qd")
            nc.vector.tensor_tensor(qd[:], b_sb[:], uT[:], OP.subtract)
            qsq = cpool.tile([BIT, B], f32, tag="qsq")
            qcol = cpool.tile([BIT, 1], f32, tag="qcol")
            nc.scalar.activation(qsq[:], qd[:], AF.Square, accum_out=qcol[:])

            # B = b @ y via transposed halves; CBVb_j = a_j*a1*B in bf16
            bT_ps = ps_z2.tile([128, BIT], f32, tag="z2")
            nc.tensor.transpose(bT_ps[:], b_sb[:, 0:128], I64[:])
            bT0 = cpool.tile([128, BIT], f32, tag="bT0")
            nc.scalar.copy(bT0[:], bT_ps[:])
            bT_ps2 = ps_z2.tile([128, BIT], f32, tag="z2")
            nc.tensor.transpose(bT_ps2[:], b_sb[:, 128:256], I64[:])
            bT1 = cpool.tile([128, BIT], f32, tag="bT1")
            nc.scalar.copy(bT1[:], bT_ps2[:])

            B_ps = ps_z1.tile([BIT, N_CLASS], f32, tag="z1")
            nc.tensor.matmul(B_ps[:], bT0[:], y0[:], start=True, stop=False)
            nc.tensor.matmul(B_ps[:], bT1[:], y1[:], start=False, stop=True)
            # CB half of the merged operand: s_al*a1*B - s_al*a3*ones in
            # CBVb-form; the K=128 merged matmul (VbsCB^T @ SbI) then yields
            # cubic + CB^T, carrying both the b@y term and the sign-trick
            # offset (sign(W) = 2*(W>0) - 1) for the whole pair.
            B_sb = cpool.tile([BIT, N_CLASS], f32, tag="B_sb")
            nc.scalar.copy(B_sb[:], B_ps[:])

            def build_cb(tile_, s_al):
                nc.scalar.activation(
                    tile_[BIT:128, :], B_sb[:], AF.Copy,
                    bias=-s_al * A3, scale=s_al * A1,
                )

            # ---- metric accumulator ----
            NT = 2 * 2 * N_CHUNKS  # 52 softplus tiles
            acc = apool.tile([128, NT], f32, tag="acc")

            metric_units = []
            for c in range(N_CHUNKS):
                cw = min(CHUNK, HALF - c * CHUNK)
                for h in range(2):
                    for m in range(2):
                        metric_units.append((c, h, m, cw))
            mu_state = {}
            stageA_done = [0]
            stageB_done = [0]
            acc_idx = [0]

            def emit_A(i):
                c, h, m, cw = metric_units[i]
                if m == 0 and h == 0:
                    if c in prefetched_u:
                        mu_state[("U", c)] = prefetched_u.pop(c)
                    else:
                        ut = upool.tile([128, CHUNK], bf16, tag="Ut")
                        cs = c * CHUNK
                        nc.sync.dma_start(
                            ut[0:64, 0:cw], d_Ush[0:64, cs : cs + cw]
                        )
                        nc.sync.dma_start(
                            ut[64:128, 0:cw], d_Ush[64:128, cs : cs + cw]
                        )
                        mu_state[("U", c)] = ut
                if m == 0:
                    if (c, h) in prefetched_y:
                        mu_state[("Y", c, h)] = prefetched_y.pop((c, h))
                    else:
                        yt = ypool.tile([N_CLASS, CHUNK], bf16, tag="Yt")
                        off = h * HALF + c * CHUNK
                        nc.sync.dma_start(
                            yt[0:50, 0:cw], d_Ysh[0:50, off : off + cw]
                        )
                        nc.sync.dma_start(
                            yt[50:100, 0:cw], d_Ysh[50:100, off : off + cw]
                        )
                        mu_state[("Y", c, h)] = yt
                ut = mu_state[("U", c)]
                yt = mu_state[("Y", c, h)]
                z1 = ps_z1.tile([128, CHUNK], f32, tag="z1")
                nc.tensor.matmul(
                    z1[:, 0:cw],
                    u05T[h * BIT : (h + 1) * BIT, m * 128 : (m + 1) * 128],
                    ut[h * BIT : (h + 1) * BIT, 0:cw],
                    start=True,
                    stop=True,
                )
                z2 = ps_z2.tile([128, CHUNK], f32, tag="z2")
                nc.tensor.matmul(
                    z2[:, 0:cw],
                    yTn2[:, m * 128 : (m + 1) * 128],
                    yt[:, 0:cw],
                    start=True,
                    stop=True,
                )
                c1 = mpool.tile([128, CHUNK], f32, tag="c1")
                if i % 3 == 2:
                    nc.vector.tensor_copy(c1[:, 0:cw], z1[:, 0:cw])
                else:
                    nc.scalar.copy(c1[:, 0:cw], z1[:, 0:cw])
                mu_state[("AB", i)] = (z1, z2, c1, cw)

            BATCH = 4
            wbig = {"t": None, "n": 0}

            def flush_wbig():
                # one wide Exp+Ln amortizes the ~352-cycle ACT pipeline fill
                n = wbig["n"]
                if n == 0:
                    return
                wt = wbig["t"]
                ew = mpool.tile([128, BATCH * CHUNK], f32, tag="ew")
                nc.scalar.activation(
                    ew[:, 0 : n * CHUNK], wt[:, 0 : n * CHUNK], AF.Exp,
                    bias=M_MARGIN,
                )
                sp = mpool.tile([128, BATCH * CHUNK], f32, tag="sp")
                i2 = acc_idx[0]
                nc.scalar.activation(
                    sp[:, 0 : n * CHUNK],
                    ew[:, 0 : n * CHUNK],
                    AF.Ln,
                    bias=1.0,
                    accum_out=acc[:, i2 : i2 + 1],
                )
                acc_idx[0] += 1
                wbig["t"] = None
                wbig["n"] = 0

            def emit_B(i):
                z1, z2, c1, cw = mu_state.pop(("AB", i))
                if cw == CHUNK:
                    if wbig["t"] is None:
                        wbig["t"] = mpool.tile([128, BATCH * CHUNK], f32, name="wb", tag="wb")
                    wt = wbig["t"]
                    o = wbig["n"] * CHUNK
                    nc.vector.scalar_tensor_tensor(
                        wt[:, o : o + cw], z2[:, 0:cw], 1.0, c1[:, 0:cw],
                        OP.add, OP.mult,
                    )
                    wbig["n"] += 1
                    if wbig["n"] == BATCH:
                        flush_wbig()
                    return
                flush_wbig()
                w = mpool.tile([128, CHUNK], f32, tag="w")
                nc.vector.scalar_tensor_tensor(
                    w[:, 0:cw], z2[:, 0:cw], 1.0, c1[:, 0:cw], OP.add, OP.mult
                )
                ew = mpool.tile([128, CHUNK], f32, tag="ew")
                nc.scalar.activation(ew[:, 0:cw], w[:, 0:cw], AF.Exp, bias=M_MARGIN)
                sp = mpool.tile([128, CHUNK], f32, tag="sp")
                i2 = acc_idx[0]
                nc.scalar.activation(
                    sp[:, 0:cw],
                    ew[:, 0:cw],
                    AF.Ln,
                    bias=1.0,
                    accum_out=acc[:, i2 : i2 + 1],
                )
                acc_idx[0] += 1

            NU = len(metric_units)

            def emit_metric(frac):
                # keep stage A one unit ahead of stage B
                wantA = min(NU, int(frac * NU) + 1)
                while stageA_done[0] < wantA:
                    emit_A(stageA_done[0])
                    stageA_done[0] += 1
                wantB = min(NU, stageA_done[0] - 1, int(frac * NU))
                while stageB_done[0] < wantB:
                    emit_B(stageB_done[0])
                    stageB_done[0] += 1

            def finish_metric():
                while stageA_done[0] < NU:
                    emit_A(stageA_done[0])
                    stageA_done[0] += 1
                while stageB_done[0] < NU:
                    emit_B(stageB_done[0])
                    stageB_done[0] += 1
                flush_wbig()

            # ---- V loop: fused iteration groups ----
            # Group k applies iters [G*k, G*k+G) in one update using state two
            # groups back: W' = W + [Ap_k @ Wb + VbsCB^T @ SbI + I @ g], where
            # the group's summed alphas are baked into the operands.
            NPAIR = N_ITERS // GROUP
            pj = [
                tuple(_alpha_idx(GROUP * k + i) for i in range(GROUP))
                for k in range(NPAIR)
            ]
            variants = []
            var_of = []
            for k in range(NPAIR):
                if pj[k] not in variants:
                    variants.append(pj[k])
                var_of.append(variants.index(pj[k]))
            assert len(variants) <= 5

            def s_alpha(k):
                return sum(ALPHAS[j] for j in pj[k])

            # prelude: CB halves for the initial variant
            build_cb(VbsCB[0], s_alpha(0))
            build_cb(VbsCB[1], s_alpha(1))
            cb_cur = [var_of[0], var_of[1]]

            W = W0
            states = {0: (W0b, G0b), 1: (W0b, G0b)}
            for k in range(NPAIR):
                s_al = s_alpha(k)
                par = k % 2
                Wb_s, g_s = states.pop(k)

                Wp = ps_w.tile([N_CLASS, BIT], f32, tag="Wp")
                nc.tensor.matmul(
                    Wp[:], Apb[var_of[k]][:], Wb_s[:], start=True, stop=False
                )
                nc.tensor.matmul(
                    Wp[:], VbsCB[par][:], SbI[par][:], start=False, stop=False
                )
                # sign term: g_s holds {0, 2*s_a3*a3-ish} pre-scaled; I-inject
                nc.tensor.matmul(
                    Wp[:], I100b[:], g_s[:], start=False, stop=True
                )

                Wn = vpool.tile([N_CLASS, BIT], f32, tag="W")
                nc.vector.scalar_tensor_tensor(
                    Wn[:], Wp[:], 1.0, W[:], OP.bypass, OP.add
                )
                W = Wn

                # refresh state + merged operands for pair k+2 (same parity)
                if k + 2 < NPAIR:
                    Wb2 = bpool.tile([N_CLASS, BIT], bf16, tag="Wb")
                    nc.gpsimd.tensor_copy(Wb2[:], Wn[:])
                    g2 = bpool.tile([N_CLASS, BIT], bf16, tag="g")
                    nc.vector.tensor_scalar(
                        g2[:], Wn[:], 0.0, 2.0 * s_alpha(k + 2) * A3,
                        OP.is_gt, OP.mult,
                    )
                    S_ps = ps_s.tile([BIT, BIT], f32, tag="S")
                    nc.tensor.matmul(S_ps[:], Wb2[:], Wb2[:], start=True, stop=True)
                    nc.vector.tensor_copy(SbI[par][0:BIT, :], S_ps[:])
                    Vt_ps = ps_v.tile([BIT, N_CLASS], bf16, tag="Vt")
                    nc.tensor.transpose(Vt_ps[:], Wb2[:], I100b[:])
                    nc.vector.tensor_scalar(
                        VbsCB[par][0:BIT, :], Vt_ps[:],
                        -s_alpha(k + 2) * A2, None, OP.mult,
                    )
                    if var_of[k + 2] != cb_cur[par]:
                        build_cb(VbsCB[par], s_alpha(k + 2))
                        cb_cur[par] = var_of[k + 2]
                    states[k + 2] = (Wb2, g2)

                # spread metric work evenly across the loop
                emit_metric((k + 1) / NPAIR)

            finish_metric()

            # ---- finalize: V_new = W^T (fp32 transpose) ----
            Vf_ps = ps_v.tile([BIT, N_CLASS], f32, tag="Vt")
            nc.tensor.transpose(Vf_ps[:], W[:], I100f[:])
            Vf = tpool.tile([BIT, N_CLASS], f32, tag="Vf")
            nc.scalar.copy(Vf[:], Vf_ps[:])
            nc.sync.dma_start(d_Vout[:], Vf[:])

            rsum = apool.tile([128, 1], f32, tag="rsum")
            nc.vector.tensor_reduce(rsum[:], acc[:], mybir.AxisListType.X, OP.add)
            part = apool.tile([1, 2], f32, tag="part")
            msum_ps = ps_s.tile([1, 1], f32, tag="S")
            nc.tensor.matmul(msum_ps[:], rsum[:], ones[:], start=True, stop=True)
            nc.scalar.copy(part[:, 0:1], msum_ps[:])
            qsum_ps = ps_s.tile([1, 1], f32, tag="S")
            nc.tensor.matmul(
                qsum_ps[:], qcol[:], ones[0:BIT, :], start=True, stop=True
            )
            nc.scalar.copy(part[:, 1:2], qsum_ps[:])
            nc.sync.dma_start(d_part[:], part[:])

    nc.compile()
    return nc


def _prep_inputs(u, y, ind, U, Y, V, TK):
    u = np.ascontiguousarray(u, dtype=np.float32)
    y = np.ascontiguousarray(y, dtype=np.float32)
    U = np.array(U, dtype=np.float32, copy=True)
    Y = np.array(Y, dtype=np.float32, copy=True)
    V = np.ascontiguousarray(V, dtype=np.float32)
    TK = np.ascontiguousarray(TK, dtype=np.float32)
    ind = np.asarray(ind)

    # scatter the batch into the train buffers (reference's .at[:, ind].set)
    U[:, ind] = u.T
    Y[:, ind] = y.T

    counts = y.sum(axis=0).astype(np.float32)
    eye = np.eye(N_CLASS, dtype=np.float32)

    def atil(j):
        al = ALPHAS[j]
        return -al * A3 * eye - al * A1 * np.diag(counts) + al * A2 * TK

    ngroup = N_ITERS // GROUP
    pj = [
        tuple(_alpha_idx(GROUP * k + i) for i in range(GROUP)) for k in range(ngroup)
    ]
    variants = []
    for k in range(ngroup):
        if pj[k] not in variants:
            variants.append(pj[k])
    Ap = [
        sum(atil(j) for j in js).astype(_bf16) for js in variants
    ]
    while len(Ap) < 5:
        Ap.append(np.zeros((N_CLASS, N_CLASS), dtype=_bf16))

    W0 = np.ascontiguousarray(V.T)
    W0b = W0.astype(_bf16)
    W0bf = W0b.astype(np.float32)
    S0b = (W0bf.T @ W0bf).astype(_bf16)
    SbI0 = np.vstack([S0b, np.eye(BIT, dtype=np.float32).astype(_bf16)])
    s_al0 = sum(ALPHAS[j] for j in pj[0])
    V0bs = (-s_al0 * A2 * W0bf.T).astype(_bf16)
    G0b = ((W0 > 0) * np.float32(2.0 * s_al0 * A3)).astype(_bf16)

    base = {
        "u05T": np.ascontiguousarray(
            np.vstack([0.5 * u.T, 0.5 * u.T]).astype(_bf16)
        ),  # duplicated so the h=1 matmul's lhsT shares rhs's base partition
        "uT": np.ascontiguousarray(u.T),
        "yT": np.ascontiguousarray(y.T),
        "yTn2": np.ascontiguousarray((-2.0 * y.T).astype(_bf16)),
        "y0": np.ascontiguousarray(y[0:128, :]),
        "y1": np.ascontiguousarray(y[128:256, :]),
        "W0": W0,
        "W0b": W0b,
        "SbI0": np.ascontiguousarray(SbI0),
        "V0bs": np.ascontiguousarray(V0bs),
        "G0b": G0b,
        "Ap0": Ap[0],
        "Ap1": Ap[1],
        "Ap2": Ap[2],
        "Ap3": Ap[3],
        "Ap4": Ap[4],
        "I64": np.eye(BIT, dtype=np.float32),
        "I100b": eye.astype(_bf16),
        "I100f": eye,
        "ones": np.ones((128, 1), dtype=np.float32),
    }

    in_maps = []
    for k in range(N_CORES):
        sl = slice(k * SHARD, (k + 1) * SHARD)
        Usl = U[:, sl]  # [64, 12500]
        Ush = Usl.reshape(BIT, 2, HALF).transpose(1, 0, 2).reshape(128, HALF)
        m = dict(base)
        m["Ush"] = np.ascontiguousarray(Ush.astype(_bf16))
        m["Ysh"] = np.ascontiguousarray(Y[:, sl].astype(_bf16))
        in_maps.append(m)
    return in_maps


def kernel(u, y, ind, U, Y, V, TK):
    from concourse.bass_utils import run_bass_kernel_spmd

    if "nc" not in _CACHE:
        _CACHE["nc"] = _build()
    nc = _CACHE["nc"]

    in_maps = _prep_inputs(u, y, ind, U, Y, V, TK)
    res = run_bass_kernel_spmd(nc, in_maps, core_ids=list(range(N_CORES)))

    metric_sum = sum(float(res.results[k]["partials"][0, 0]) for k in range(N_CORES))
    quant_sum = float(res.results[0]["partials"][0, 1])
    loss = metric_sum / (B * NUM_TRAIN) + ETA * quant_sum / (BIT * B)
    V_new = np.asarray(res.results[0]["V_new"], dtype=np.float32)
    return np.float32(loss), V_new


# revision 38
# speedup vs baseline: 1.0137x; 1.0059x over previous
"""Trainium2 Bass kernel for nn_DFHLoss (DFH loss_fn).

Computes, on 8 NeuronCores (data-parallel over num_train):
  - metric loss: mean over [256, 100000] of softplus pairwise terms
  - 200-step inner codebook SGD on V [64,100] (replicated on every core)
  - quantization loss
Returns (loss_scalar, V_new) matching the jax reference.

Math restructuring (validated vs reference, V rel err ~5e-4 across seeds):
  The SGD step folds to W' = W + a*a3*sign(W) + [At_j @ W + (-a*a2)(W S) + CBW]
  with W = V^T, S = V V^T, At_j = A_j - I small (~1e-3), CBW = a*a1*(b@y)^T.
  The fp32 carry (W + ...) rides the DVE add; every matmul term is a small
  correction, so all matmuls run bf16. The bracketed terms consume state
  (Wb/Sb/Vbs/sgn) derived from W_{t-1} (one-step stale), which removes the
  matmul chain from the serial critical path; staleness shifts V_new by
  ~2e-4 relative, far under tolerance.
  The metric elementwise chain folds to softplus((z2'+1)*z1' + 1) with
  z1' = (0.5u)@U, z2' = (-2y)@Y (both bf16: error averages out over 25.6M
  terms, ~4e-7 on the mean; one-hot z2' is exact). The reference's
  clip(ip,-100,50) only binds on s=1 self-pairs where softplus(M-ip)~1e-22,
  numerically irrelevant at fp32, so it is elided. softplus = Ln(Exp(w)+1)
  on the scalar engine (both funcs share one ACT table) with a fused
  per-partition accumulator on the Ln pass.
"""

import numpy as np
import ml_dtypes

_bf16 = ml_dtypes.bfloat16

N_CORES = 8
NUM_TRAIN = 100000
SHARD = NUM_TRAIN // N_CORES  # 12500
HALF = SHARD // 2  # 6250
B = 256
BIT = 64
N_CLASS = 100
MU, M_MARGIN, ETA, VUL, NTA = 1.0, 1.0, 0.5, 1.0, 1.0

A1 = 2.0 / (BIT * B)
A2 = VUL * 4.0 / (N_CLASS * N_CLASS)
A3 = NTA * 2.0 / (BIT * N_CLASS)

ALPHAS = [0.03, 0.003, 0.0003]
N_ITERS = 200
GROUP = 5  # iterations fused per update (validated ~3-4e-3 vs reference)


def _alpha_idx(t):
    if t >= 179:
        return 2
    if t >= 149:
        return 1
    return 0


CHUNK = 512
N_CHUNKS = (HALF + CHUNK - 1) // CHUNK  # 13 (last chunk 106 cols)

_CACHE = {}


def _build():
    import concourse.bacc as bacc
    import concourse.tile as tile
    import concourse.mybir as mybir

    f32 = mybir.dt.float32
    bf16 = mybir.dt.bfloat16
    AF = mybir.ActivationFunctionType
    OP = mybir.AluOpType

    # Force a single ACT function table: the default greedy table chooser
    # ping-pongs between 'exp_and_others' and 'natural_log' (one ~1.3us
    # ACT_TABLE_LOAD per metric tile). Emptying every table except the one
    # containing ALL funcs we use (exp, ln, sign, copy, square, identity)
    # keeps positions (= act_func_set_id) valid while making the chooser
    # always pick it.
    from concourse.hw_specs import get_activation_tables as _orig_gat

    def _one_table(arch):
        tabs = _orig_gat(arch)
        return {
            name: (funcs if name == "natural_log_exp_and_others" else frozenset())
            for name, funcs in tabs.items()
        }

    bacc.get_activation_tables = _one_table

    nc = bacc.Bacc(None, target_bir_lowering=False)

    # inputs (identical on every core except Ush/Ysh)
    d_u05T = nc.declare_dram_parameter("u05T", [128, B], bf16, isOutput=False)
    d_uT = nc.declare_dram_parameter("uT", [BIT, B], f32, isOutput=False)
    d_yT = nc.declare_dram_parameter("yT", [N_CLASS, B], f32, isOutput=False)
    d_yTn2 = nc.declare_dram_parameter("yTn2", [N_CLASS, B], bf16, isOutput=False)
    d_y0 = nc.declare_dram_parameter("y0", [128, N_CLASS], f32, isOutput=False)
    d_y1 = nc.declare_dram_parameter("y1", [128, N_CLASS], f32, isOutput=False)
    d_W0 = nc.declare_dram_parameter("W0", [N_CLASS, BIT], f32, isOutput=False)
    d_W0b = nc.declare_dram_parameter("W0b", [N_CLASS, BIT], bf16, isOutput=False)
    d_SbI0 = nc.declare_dram_parameter("SbI0", [128, BIT], bf16, isOutput=False)
    d_V0bs = nc.declare_dram_parameter("V0bs", [BIT, N_CLASS], bf16, isOutput=False)
    d_G0b = nc.declare_dram_parameter("G0b", [N_CLASS, BIT], bf16, isOutput=False)
    d_Ap = [
        nc.declare_dram_parameter(f"Ap{v}", [N_CLASS, N_CLASS], bf16, isOutput=False)
        for v in range(5)
    ]
    d_I64 = nc.declare_dram_parameter("I64", [BIT, BIT], f32, isOutput=False)
    d_I100b = nc.declare_dram_parameter(
        "I100b", [N_CLASS, N_CLASS], bf16, isOutput=False
    )
    d_I100f = nc.declare_dram_parameter(
        "I100f", [N_CLASS, N_CLASS], f32, isOutput=False
    )
    d_ones = nc.declare_dram_parameter("ones", [128, 1], f32, isOutput=False)
    d_Ush = nc.declare_dram_parameter("Ush", [128, HALF], bf16, isOutput=False)
    d_Ysh = nc.declare_dram_parameter("Ysh", [N_CLASS, SHARD], bf16, isOutput=False)

    # outputs
    d_Vout = nc.declare_dram_parameter("V_new", [BIT, N_CLASS], f32, isOutput=True)
    d_part = nc.declare_dram_parameter("partials", [1, 2], f32, isOutput=True)

    prefetched_u = {}
    prefetched_y = {}
    with tile.TileContext(nc) as tc:
        with (
            tc.tile_pool(name="consts", bufs=1) as cpool,
            tc.tile_pool(name="wstate", bufs=3) as vpool,
            tc.tile_pool(name="bstate", bufs=4) as bpool,
            tc.tile_pool(name="vtmp", bufs=2) as tpool,
            tc.tile_pool(name="uchunk", bufs=4) as upool,
            tc.tile_pool(name="ychunk", bufs=4) as ypool,
            tc.tile_pool(name="mscratch", bufs=3) as mpool,
            tc.tile_pool(name="acc", bufs=1) as apool,
            tc.tile_pool(name="ps_z1", bufs=2, space="PSUM") as ps_z1,
            tc.tile_pool(name="ps_z2", bufs=2, space="PSUM") as ps_z2,
            tc.tile_pool(name="ps_w", bufs=2, space="PSUM") as ps_w,
            tc.tile_pool(name="ps_s", bufs=1, space="PSUM") as ps_s,
            tc.tile_pool(name="ps_v", bufs=1, space="PSUM") as ps_v,
        ):
            # ---- prefetch the first metric chunks (sync queues) while the
            # constants load on the gpsimd queues: kills the startup bubble
            for c0 in (0, 1):
                ut = upool.tile([128, CHUNK], bf16, tag="Ut")
                cs = c0 * CHUNK
                for q in range(4):
                    p0, p1 = 32 * q, 32 * (q + 1)
                    nc.sync.dma_start(
                        ut[p0:p1, :], d_Ush[p0:p1, cs : cs + CHUNK]
                    )
                prefetched_u[c0] = ut
                for h0 in (0, 1):
                    yt = ypool.tile([N_CLASS, CHUNK], bf16, tag="Yt")
                    off = h0 * HALF + c0 * CHUNK
                    for q in range(4):
                        p0, p1 = 25 * q, 25 * (q + 1)
                        nc.sync.dma_start(
                            yt[p0:p1, :], d_Ysh[p0:p1, off : off + CHUNK]
                        )
                    prefetched_y[(c0, h0)] = yt

            # ---- load constants ----
            def cload(dram, shape, tag, dt=f32):
                t = cpool.tile(shape, dt, tag=tag)
                nc.gpsimd.dma_start(t[:], dram[:])
                return t

            # order matters: the gpsimd issue thread serializes these, so
            # prelude-critical tensors go first, loop/finalize tensors last
            yT = cload(d_yT, [N_CLASS, B], "yT")
            uT = cload(d_uT, [BIT, B], "uT")
            W0 = cload(d_W0, [N_CLASS, BIT], "W0")
            I64 = cload(d_I64, [BIT, BIT], "I64")
            y0 = cload(d_y0, [128, N_CLASS], "y0")
            y1 = cload(d_y1, [128, N_CLASS], "y1")
            u05T = cload(d_u05T, [128, B], "u05T", bf16)
            yTn2 = cload(d_yTn2, [N_CLASS, B], "yTn2", bf16)
            W0b = cload(d_W0b, [N_CLASS, BIT], "W0b", bf16)
            G0b = cload(d_G0b, [N_CLASS, BIT], "G0b", bf16)
            Apb = [cload(d_Ap[v], [N_CLASS, N_CLASS], f"Ap{v}", bf16) for v in range(5)]
            I100b = cload(d_I100b, [N_CLASS, N_CLASS], "I100b", bf16)
            I100f = cload(d_I100f, [N_CLASS, N_CLASS], "I100f")
            ones = cload(d_ones, [128, 1], "ones")
            # ping-pong merged operands: SbI = [Sb ; I64], VbsCB = [Vbs ; CBVb]
            SbI = []
            VbsCB = []
            for p in range(2):
                t = cpool.tile([128, BIT], bf16, tag=f"SbI{p}")
                nc.gpsimd.dma_start(t[:], d_SbI0[:])
                SbI.append(t)
                t2 = cpool.tile([128, N_CLASS], bf16, tag=f"VbsCB{p}")
                nc.gpsimd.dma_start(t2[0:BIT, :], d_V0bs[:])
                VbsCB.append(t2)
         # ---- prelude: b = sign(C @ yT + uT), B = b@y, CBV (bf16) ----
            sgW0 = cpool.tile([N_CLASS, BIT], f32, tag="sgW0")
            nc.scalar.activation(sgW0[:], W0[:], AF.Sign)
            b_ps = ps_z1.tile([BIT, B], f32, tag="z1")
            nc.tensor.matmul(b_ps[:], sgW0[:], yT[:], start=True, stop=True)
            badd = cpool.tile([BIT, B], f32, tag="badd")
            nc.vector.tensor_tensor(badd[:], b_ps[:], uT[:], OP.add)
            b_sb = cpool.tile([BIT, B], f32, tag="b_sb")
            nc.scalar.activation(b_sb[:], badd[:], AF.Sign)

            # quant: sum((b - uT)^2)
            qd = cpool.tile([BIT, B], f32, tag="qd")
            nc.vector.tensor_tensor(qd[:], b_sb[:], uT[:], OP.subtract)
            qsq = cpool.tile([BIT, B], f32, tag="qsq")
            qcol = cpool.tile([BIT, 1], f32, tag="qcol")
            nc.scalar.activation(qsq[:], qd[:], AF.Square, accum_out=qcol[:])

            # B = b @ y via transposed halves; CBVb_j = a_j*a1*B in bf16
            bT_ps = ps_z2.tile([128, BIT], f32, tag="z2")
            nc.tensor.transpose(bT_ps[:], b_sb[:, 0:128], I64[:])
            bT0 = cpool.tile([128, BIT], f32, tag="bT0")
            nc.scalar.copy(bT0[:], bT_ps[:])
            bT_ps2 = ps_z2.tile([128, BIT], f32, tag="z2")
            nc.tensor.transpose(bT_ps2[:], b_sb[:, 128:256], I64[:])
            bT1 = cpool.tile([128, BIT], f32, tag="bT1")
            nc.scalar.copy(bT1[:], bT_ps2[:])

            B_ps = ps_z1.tile([BIT, N_CLASS], f32, tag="z1")
            nc.tensor.matmul(B_ps[:], bT0[:], y0[:], start=True, stop=False)
            nc.tensor.matmul(B_ps[:], bT1[:], y1[:], start=False, stop=True)
            # CB half of the merged operand: s_al*a1*B - s_al*a3*ones in
            # CBVb-form; the K=128 merged matmul (VbsCB^T @ SbI) then yields
            # cubic + CB^T, carrying both the b@y term and the sign-trick
            # offset (sign(W) = 2*(W>0) - 1) for the whole pair.
            B_sb = cpool.tile([BIT, N_CLASS], f32, tag="B_sb")
            nc.scalar.copy(B_sb[:], B_ps[:])

            def build_cb(tile_, s_al):
                nc.scalar.activation(
                    tile_[BIT:128, :], B_sb[:], AF.Copy,
                    bias=-s_al * A3, scale=s_al * A1,
                )

            # ---- metric accumulator ----
            NT = 2 * 2 * N_CHUNKS  # 52 softplus tiles
            acc = apool.tile([128, NT], f32, tag="acc")

            metric_units = []
            for c in range(N_CHUNKS):
                cw = min(CHUNK, HALF - c * CHUNK)
                for h in range(2):
                    for m in range(2):
                        metric_units.append((c, h, m, cw))
            mu_state = {}
            stageA_done = [0]
            stageB_done = [0]
            acc_idx = [0]

            def emit_A(i):
                c, h, m, cw = metric_units[i]
                if m == 0 and h == 0:
                    if c in prefetched_u:
                        mu_state[("U", c)] = prefetched_u.pop(c)
                    else:
                        ut = upool.tile([128, CHUNK], bf16, tag="Ut")
                        cs = c * CHUNK
                        nc.sync.dma_start(
                            ut[0:64, 0:cw], d_Ush[0:64, cs : cs + cw]
                        )
                        nc.sync.dma_start(
                            ut[64:128, 0:cw], d_Ush[64:128, cs : cs + cw]
                        )
                        mu_state[("U", c)] = ut
                if m == 0:
                    if (c, h) in prefetched_y:
                        mu_state[("Y", c, h)] = prefetched_y.pop((c, h))
                    else:
                        yt = ypool.tile([N_CLASS, CHUNK], bf16, tag="Yt")
                        off = h * HALF + c * CHUNK
                        nc.sync.dma_start(
                            yt[0:50, 0:cw], d_Ysh[0:50, off : off + cw]
                        )
                        nc.sync.dma_start(
                            yt[50:100, 0:cw], d_Ysh[50:100, off : off + cw]
                        )
                        mu_state[("Y", c, h)] = yt
                ut = mu_state[("U", c)]
                yt = mu_state[("Y", c, h)]
                z1 = ps_z1.tile([128, CHUNK], f32, tag="z1")
                nc.tensor.matmul(
                    z1[:, 0:cw],
                    u05T[h * BIT : (h + 1) * BIT, m * 128 : (m + 1) * 128],
                    ut[h * BIT : (h + 1) * BIT, 0:cw],
                    start=True,
                    stop=True,
                )
                z2 = ps_z2.tile([128, CHUNK], f32, tag="z2")
                nc.tensor.matmul(
                    z2[:, 0:cw],
                    yTn2[:, m * 128 : (m + 1) * 128],
                    yt[:, 0:cw],
                    start=True,
                    stop=True,
                )
                c1 = mpool.tile([128, CHUNK], f32, tag="c1")
                if i % 3 == 2:
                    nc.vector.tensor_copy(c1[:, 0:cw], z1[:, 0:cw])
                else:
                    nc.scalar.copy(c1[:, 0:cw], z1[:, 0:cw])
                mu_state[("AB", i)] = (z1, z2, c1, cw)

            BATCH = 4
            wbig = {"t": None, "n": 0}

            def flush_wbig():
                # one wide Exp+Ln amortizes the ~352-cycle ACT pipeline fill
                n = wbig["n"]
                if n == 0:
                    return
                wt = wbig["t"]
                ew = mpool.tile([128, BATCH * CHUNK], f32, tag="ew")
                nc.scalar.activation(
                    ew[:, 0 : n * CHUNK], wt[:, 0 : n * CHUNK], AF.Exp,
                    bias=M_MARGIN,
                )
                sp = mpool.tile([128, BATCH * CHUNK], f32, tag="sp")
                i2 = acc_idx[0]
                nc.scalar.activation(
                    sp[:, 0 : n * CHUNK],
                    ew[:, 0 : n * CHUNK],
                    AF.Ln,
                    bias=1.0,
                    accum_out=acc[:, i2 : i2 + 1],
                )
                acc_idx[0] += 1
                wbig["t"] = None
                wbig["n"] = 0

            def emit_B(i):
                z1, z2, c1, cw = mu_state.pop(("AB", i))
                if cw == CHUNK:
                    if wbig["t"] is None:
                        wbig["t"] = mpool.tile([128, BATCH * CHUNK], f32, name="wb", tag="wb")
                    wt = wbig["t"]
                    o = wbig["n"] * CHUNK
                    nc.vector.scalar_tensor_tensor(
                        wt[:, o : o + cw], z2[:, 0:cw], 1.0, c1[:, 0:cw],
                        OP.add, OP.mult,
                    )
                    wbig["n"] += 1
                    if wbig["n"] == BATCH:
                        flush_wbig()
                    return
                flush_wbig()
                w = mpool.tile([128, CHUNK], f32, tag="w")
                nc.vector.scalar_tensor_tensor(
                    w[:, 0:cw], z2[:, 0:cw], 1.0, c1[:, 0:cw], OP.add, OP.mult
                )
                ew = mpool.tile([128, CHUNK], f32, tag="ew")
                nc.scalar.activation(ew[:, 0:cw], w[:, 0:cw], AF.Exp, bias=M_MARGIN)
                sp = mpool.tile([128, CHUNK], f32, tag="sp")
                i2 = acc_idx[0]
                nc.scalar.activation(
                    sp[:, 0:cw],
                    ew[:, 0:cw],
                    AF.Ln,
                    bias=1.0,
                    accum_out=acc[:, i2 : i2 + 1],
                )
                acc_idx[0] += 1

            NU = len(metric_units)

            def emit_metric(frac):
                # keep stage A one unit ahead of stage B
                wantA = min(NU, int(frac * NU) + 1)
                while stageA_done[0] < wantA:
                    emit_A(stageA_done[0])
                    stageA_done[0] += 1
                wantB = min(NU, stageA_done[0] - 1, int(frac * NU))
                while stageB_done[0] < wantB:
                    emit_B(stageB_done[0])
                    stageB_done[0] += 1

            def finish_metric():
                while stageA_done[0] < NU:
                    emit_A(stageA_done[0])
                    stageA_done[0] += 1
                while stageB_done[0] < NU:
                    emit_B(stageB_done[0])
                    stageB_done[0] += 1
                flush_wbig()

            # ---- V loop: fused iteration groups ----
            # Group k applies iters [G*k, G*k+G) in one update using state two
            # groups back: W' = W + [Ap_k @ Wb + VbsCB^T @ SbI + I @ g], where
            # the group's summed alphas are baked into the operands.
            NPAIR = N_ITERS // GROUP
            pj = [
                tuple(_alpha_idx(GROUP * k + i) for i in range(GROUP))
                for k in range(NPAIR)
            ]
            variants = []
            var_of = []
            for k in range(NPAIR):
                if pj[k] not in variants:
                    variants.append(pj[k])
                var_of.append(variants.index(pj[k]))
            assert len(variants) <= 5

            def s_alpha(k):
                return sum(ALPHAS[j] for j in pj[k])

            # prelude: CB halves for the initial variant
            build_cb(VbsCB[0], s_alpha(0))
            build_cb(VbsCB[1], s_alpha(1))
            cb_cur = [var_of[0], var_of[1]]

            W = W0
            states = {0: (W0b, G0b), 1: (W0b, G0b)}
            for k in range(NPAIR):
                s_al = s_alpha(k)
                par = k % 2
                Wb_s, g_s = states.pop(k)

                Wp = ps_w.tile([N_CLASS, BIT], f32, tag="Wp")
                nc.tensor.matmul(
                    Wp[:], Apb[var_of[k]][:], Wb_s[:], start=True, stop=False
                )
                nc.tensor.matmul(
                    Wp[:], VbsCB[par][:], SbI[par][:], start=False, stop=False
                )
                # sign term: g_s holds {0, 2*s_a3*a3-ish} pre-scaled; I-inject
                nc.tensor.matmul(
                    Wp[:], I100b[:], g_s[:], start=False, stop=True
                )

                Wn = vpool.tile([N_CLASS, BIT], f32, tag="W")
                nc.vector.scalar_tensor_tensor(
                    Wn[:], Wp[:], 1.0, W[:], OP.bypass, OP.add
                )
                W = Wn

                # refresh state + merged operands for pair k+2 (same parity)
                if k + 2 < NPAIR:
                    Wb2 = bpool.tile([N_CLASS, BIT], bf16, tag="Wb")
                    nc.gpsimd.tensor_copy(Wb2[:], Wn[:])
                    g2 = bpool.tile([N_CLASS, BIT], bf16, tag="g")
                    nc.vector.tensor_scalar(
                        g2[:], Wn[:], 0.0, 2.0 * s_alpha(k + 2) * A3,
                        OP.is_gt, OP.mult,
                    )
                    S_ps = ps_s.tile([BIT, BIT], f32, tag="S")
                    nc.tensor.matmul(S_ps[:], Wb2[:], Wb2[:], start=True, stop=True)
                    nc.vector.tensor_copy(SbI[par][0:BIT, :], S_ps[:])
                    Vt_ps = ps_v.tile([BIT, N_CLASS], bf16, tag="Vt")
                    nc.tensor.transpose(Vt_ps[:], Wb2[:], I100b[:])
                    nc.vector.tensor_scalar(
                        VbsCB[par][0:BIT, :], Vt_ps[:],
                        -s_alpha(k + 2) * A2, None, OP.mult,
                    )
                    if var_of[k + 2] != cb_cur[par]:
                        build_cb(VbsCB[par], s_alpha(k + 2))
                        cb_cur[par] = var_of[k + 2]
                    states[k + 2] = (Wb2, g2)

                # spread metric work evenly across the loop
                emit_metric((k + 1) / NPAIR)

            finish_metric()

            # ---- finalize: V_new = W^T (fp32 transpose) ----
            Vf_ps = ps_v.tile([BIT, N_CLASS], f32, tag="Vt")
            nc.tensor.transpose(Vf_ps[:], W[:], I100f[:])
            Vf = tpool.tile([BIT, N_CLASS], f32, tag="Vf")
            nc.scalar.copy(Vf[:], Vf_ps[:])
            nc.sync.dma_start(d_Vout[:], Vf[:])

            rsum = apool.tile([128, 1], f32, tag="rsum")
            # only acc columns [0, acc_idx) were written (batched Ln accum);
            # reducing further would sum uninitialized SBUF
            nc.vector.tensor_reduce(
                rsum[:], acc[:, 0 : acc_idx[0]], mybir.AxisListType.X, OP.add
            )
            part = apool.tile([1, 2], f32, tag="part")
            msum_ps = ps_s.tile([1, 1], f32, tag="S")
            nc.tensor.matmul(msum_ps[:], rsum[:], ones[:], start=True, stop=True)
            nc.scalar.copy(part[:, 0:1], msum_ps[:])
            qsum_ps = ps_s.tile([1, 1], f32, tag="S")
            nc.tensor.matmul(
                qsum_ps[:], qcol[:], ones[0:BIT, :], start=True, stop=True
            )
            nc.scalar.copy(part[:, 1:2], qsum_ps[:])
            nc.sync.dma_start(d_part[:], part[:])

    nc.compile()
    return nc


def _prep_inputs(u, y, ind, U, Y, V, TK):
    u = np.ascontiguousarray(u, dtype=np.float32)
    y = np.ascontiguousarray(y, dtype=np.float32)
    U = np.array(U, dtype=np.float32, copy=True)
    Y = np.array(Y, dtype=np.float32, copy=True)
    V = np.ascontiguousarray(V, dtype=np.float32)
    TK = np.ascontiguousarray(TK, dtype=np.float32)
    ind = np.asarray(ind)

    # scatter the batch into the train buffers (reference's .at[:, ind].set)
    U[:, ind] = u.T
    Y[:, ind] = y.T

    counts = y.sum(axis=0).astype(np.float32)
    eye = np.eye(N_CLASS, dtype=np.float32)

    def atil(j):
        al = ALPHAS[j]
        return -al * A3 * eye - al * A1 * np.diag(counts) + al * A2 * TK

    ngroup = N_ITERS // GROUP
    pj = [
        tuple(_alpha_idx(GROUP * k + i) for i in range(GROUP)) for k in range(ngroup)
    ]
    variants = []
    for k in range(ngroup):
        if pj[k] not in variants:
            variants.append(pj[k])
    Ap = [
        sum(atil(j) for j in js).astype(_bf16) for js in variants
    ]
    while len(Ap) < 5:
        Ap.append(np.zeros((N_CLASS, N_CLASS), dtype=_bf16))

    W0 = np.ascontiguousarray(V.T)
    W0b = W0.astype(_bf16)
    W0bf = W0b.astype(np.float32)
    S0b = (W0bf.T @ W0bf).astype(_bf16)
    SbI0 = np.vstack([S0b, np.eye(BIT, dtype=np.float32).astype(_bf16)])
    s_al0 = sum(ALPHAS[j] for j in pj[0])
    V0bs = (-s_al0 * A2 * W0bf.T).astype(_bf16)
    G0b = ((W0 > 0) * np.float32(2.0 * s_al0 * A3)).astype(_bf16)

    base = {
        "u05T": np.ascontiguousarray(
            np.vstack([0.5 * u.T, 0.5 * u.T]).astype(_bf16)
        ),  # duplicated so the h=1 matmul's lhsT shares rhs's base partition
        "uT": np.ascontiguousarray(u.T),
        "yT": np.ascontiguousarray(y.T),
        "yTn2": np.ascontiguousarray((-2.0 * y.T).astype(_bf16)),
        "y0": np.ascontiguousarray(y[0:128, :]),
        "y1": np.ascontiguousarray(y[128:256, :]),
        "W0": W0,
        "W0b": W0b,
        "SbI0": np.ascontiguousarray(SbI0),
        "V0bs": np.ascontiguousarray(V0bs),
        "G0b": G0b,
        "Ap0": Ap[0],
        "Ap1": Ap[1],
        "Ap2": Ap[2],
        "Ap3": Ap[3],
        "Ap4": Ap[4],
        "I64": np.eye(BIT, dtype=np.float32),
        "I100b": eye.astype(_bf16),
        "I100f": eye,
        "ones": np.ones((128, 1), dtype=np.float32),
    }

    in_maps = []
    for k in range(N_CORES):
        sl = slice(k * SHARD, (k + 1) * SHARD)
        Usl = U[:, sl]  # [64, 12500]
        Ush = Usl.reshape(BIT, 2, HALF).transpose(1, 0, 2).reshape(128, HALF)
        m = dict(base)
        m["Ush"] = np.ascontiguousarray(Ush.astype(_bf16))
        m["Ysh"] = np.ascontiguousarray(Y[:, sl].astype(_bf16))
        in_maps.append(m)
    return in_maps


def kernel(u, y, ind, U, Y, V, TK):
    from concourse.bass_utils import run_bass_kernel_spmd

    if "nc" not in _CACHE:
        _CACHE["nc"] = _build()
    nc = _CACHE["nc"]

    in_maps = _prep_inputs(u, y, ind, U, Y, V, TK)
    res = run_bass_kernel_spmd(nc, in_maps, core_ids=list(range(N_CORES)))

    metric_sum = sum(float(res.results[k]["partials"][0, 0]) for k in range(N_CORES))
    quant_sum = float(res.results[0]["partials"][0, 1])
    loss = metric_sum / (B * NUM_TRAIN) + ETA * quant_sum / (BIT * B)
    V_new = np.asarray(res.results[0]["V_new"], dtype=np.float32)
    return np.float32(loss), V_new
